# revision 3
# baseline (speedup 1.0000x reference)
"""Trainium2 Bass kernel for the AutoCorrelation module (Autoformer-style).

Shapes (hardcoded): B=8, N=128, L=192, H=8, E=64, D=64.

Math: for each (b, n):
  corr-mean  c[tau] = sum_s <Q_{(s+tau)%L}, K_s>  over the flattened (h,e) dim
             = circular-diagonal sums of the Gram matrix G[s,u] = <K_s, Q_u>
  top-5 delays per node from batch-averaged c (host), softmax weights (host),
  output o[tau, hd]  = sum_j w_j * v[(tau+d_j)%L, hd]
                     = (A @ V)[tau, hd]  with the sparse shift-matrix A (host-built)

Device work (8 cores, node axis sharded, 16 nodes/core, all 8 batches local):
  kernel 1: per-(b,n) Gram matrices, single-product fp16 with fp16 G store
            (measured on the real inputs: 1.8e-4 output rel err, zero top-5
            index flips vs the fp32 pipeline -- the hi/lo decomposition is
            not needed at the 2e-2 gate)
  kernel 2: per-(b,n) V^T-stationary shift-matrix matmul in fp16
Host work: transposes, diag-sums, top-k, softmax, A-matrix build, reassembly.
All DMA layouts keep >=1536B contiguous runs on BOTH sides of each descriptor
(the sub-512B runs were what held the old agg kernel at 249 GB/s).
"""

import numpy as np

import concourse.bass as bass  # noqa: F401
import concourse.mybir as mybir
import concourse.tile as tile
from concourse import bacc

B, N, L, H, E, D = 8, 128, 192, 8, 64, 64
HE = H * E            # 512
HD = H * D            # 512
NCORES = 8
NLOC = N // NCORES    # 16 nodes per core
BN = B * NLOC         # 128 (b, n) pairs per core
TOPK = 5              # int(log(192))

F32 = mybir.dt.float32
F16 = mybir.dt.float16


def _build_corr_nc(bn_count=BN, num_devices=NCORES, group=4):
    """Per (b,n): G[s,u] = sum_d k[s,d]*q[u,d], fp16 multiply / fp32 PSUM
    accumulate, G stored fp16.

    Input kqx[bn, t, p, x] fp16 (t: 0=k, 1=q; x = c*192 + l packs the d-chunk
    c (d = c*128 + p) and time l -> 1536B contiguous per (bn, t, p)).
    Output g5[quad, s, b4, u] fp16 (1536B runs per (quad, s)).
    bn's are processed in groups of 4 with one input DMA per group,
    alternating between the Sync and Scalar HWDGE rings.
    """
    nc = bacc.Bacc(
        "TRN2",
        target_bir_lowering=False,
        debug=False,
        enable_asserts=False,
        num_devices=num_devices,
    )
    kqx = nc.dram_tensor(
        "kqx", [bn_count, 2, 128, 768], F16, kind="ExternalInput"
    ).ap()
    g5 = nc.dram_tensor(
        "g5", [bn_count // 4, L, 4, L], F16, kind="ExternalOutput"
    ).ap()

    assert bn_count % group == 0
    with tile.TileContext(nc) as tc:
        with (
            tc.tile_pool(name="kin", bufs=3) as kpool,
            tc.tile_pool(name="gout", bufs=3) as gpool,
            tc.tile_pool(name="ps", bufs=2 * group, space="PSUM") as pspool,
        ):
            for qd in range(bn_count // group):
                g0 = qd * group
                kqtile = kpool.tile([128, group, 2, 768], F16)
                eng = nc.sync if qd % 2 == 0 else nc.scalar
                eng.dma_start(
                    out=kqtile[:],
                    in_=kqx[g0 : g0 + group].rearrange("b t p x -> p b t x"),
                )

                gtile = gpool.tile([128, group, 2 * L], F16)
                for i in range(group):
                    ps = pspool.tile([128, 2 * L], F32)
                    # m-chunks: G rows [0:128] -> ps[:, 0:L]; [128:192] -> ps[0:64, L:]
                    for msl, osl in (
                        (slice(0, 128), slice(0, L)),
                        (slice(128, 192), slice(L, 2 * L)),
                    ):
                        mlen = msl.stop - msl.start
                        for c in range(4):
                            x0 = c * L
                            nc.tensor.matmul(
                                ps[0:mlen, osl],
                                lhsT=kqtile[:, i, 0, x0 + msl.start : x0 + msl.stop],
                                rhs=kqtile[:, i, 1, x0 : x0 + L],
                                start=(c == 0),
                                stop=(c == 3),
                            )
                    nc.vector.tensor_copy(gtile[0:128, i, 0:L], ps[0:128, 0:L])
                    nc.scalar.copy(gtile[0:64, i, L : 2 * L], ps[0:64, L : 2 * L])

                # outputs on the SWDGE (gpsimd) ring; inputs own both HWDGE rings
                nc.gpsimd.dma_start(out=g5[qd, 0:128], in_=gtile[0:128, :, 0:L])
                nc.gpsimd.dma_start(
                    out=g5[qd, 128:192], in_=gtile[0:64, :, L : 2 * L]
                )

    nc.compile()
    return nc


def _build_agg_nc(bn_count=BN, num_devices=NCORES):
    """Per (b,n): o[hd, tau] = sum_t' v[t', hd] * at[t', tau], fp16 in/out.

    V is the stationary operand (full hd-chunk columns), AT the moving one;
    output is hd-major and transposed back on the host. Tile free-dim order
    is (kc, b4, .) so every DMA descriptor is a 1536B/4KB contiguous run on
    both the DRAM and SBUF side.
    """
    nc = bacc.Bacc(
        "TRN2",
        target_bir_lowering=False,
        debug=False,
        enable_asserts=False,
        num_devices=num_devices,
    )
    assert bn_count % 4 == 0
    nquad = bn_count // 4
    # at6[quad, kc, p, b4, t]: t' = kc*96 + p
    at6 = nc.dram_tensor(
        "at6", [nquad, 2, 96, 4, L], F16, kind="ExternalInput"
    ).ap()
    # v6[quad, kc, p, b4, d]
    v6 = nc.dram_tensor(
        "v6", [nquad, 2, 96, 4, HD], F16, kind="ExternalInput"
    ).ap()
    # o6[quad, c, p, b4, l]: output element (bn, hd=c*128+p, tau=l)
    o6 = nc.dram_tensor(
        "o6", [nquad, 4, 128, 4, L], F16, kind="ExternalOutput"
    ).ap()

    with tile.TileContext(nc) as tc:
        with (
            tc.tile_pool(name="ain", bufs=3) as apool,
            tc.tile_pool(name="vin", bufs=3) as vpool,
            tc.tile_pool(name="oout", bufs=3) as opool,
            tc.tile_pool(name="ps", bufs=8, space="PSUM") as pspool,
        ):
            for qd in range(nquad):
                # alternate input rings so each HWDGE ring carries ~half
                ea, ev = (nc.scalar, nc.sync) if qd % 2 == 0 else (nc.sync, nc.scalar)
                atile = apool.tile([96, 2, 4, L], F16)
                ea.dma_start(
                    out=atile[:], in_=at6[qd].rearrange("kc p b t -> p kc b t")
                )
                vtile = vpool.tile([96, 2, 4, HD], F16)
                ev.dma_start(
                    out=vtile[:], in_=v6[qd].rearrange("kc p b d -> p kc b d")
                )

                # otile free layout: (half, cc, b4, l); hd-chunk c = half*2+cc
                otile = opool.tile([128, 2, 2, 4, L], F16)
                for i in range(4):
                    pss = [
                        pspool.tile([128, 2 * L], F32, name="ps", tag="ps")
                        for _ in range(2)
                    ]
                    for c in range(4):
                        ps = pss[c // 2][0:128, (c % 2) * L : (c % 2 + 1) * L]
                        for kc in range(2):
                            nc.tensor.matmul(
                                ps,
                                lhsT=vtile[:, kc, i, c * 128 : (c + 1) * 128],
                                rhs=atile[:, kc, i, :],
                                start=(kc == 0),
                                stop=(kc == 1),
                            )
                    nc.vector.tensor_copy(
                        otile[:, 0, :, i, :],
                        pss[0][:].rearrange("p (cc l) -> p cc l", cc=2),
                    )
                    nc.scalar.copy(
                        otile[:, 1, :, i, :],
                        pss[1][:].rearrange("p (cc l) -> p cc l", cc=2),
                    )

                nc.gpsimd.dma_start(
                    out=o6[qd].rearrange("(half cc) p b l -> p half cc b l", half=2, cc=2),
                    in_=otile[:],
                )

    nc.compile()
    return nc


_NC_CACHE = {}


def _get_nc(name):
    if name not in _NC_CACHE:
        _NC_CACHE[name] = {"corr": _build_corr_nc, "agg": _build_agg_nc}[name]()
    return _NC_CACHE[name]


_JIT_CACHE = {}


def _run_spmd(nc, in_maps):
    """run_bass_kernel_spmd's axon path with the jitted executable cached
    per-module, so repeat kernel() calls don't re-trace/re-compile."""
    import jax
    import numpy as _np
    from jax.experimental.shard_map import shard_map
    from jax.sharding import Mesh, PartitionSpec

    from concourse import bass2jax

    key = id(nc)
    if key not in _JIT_CACHE:
        bass2jax.install_neuronx_cc_hook()
        partition_name = (
            nc.partition_id_tensor.name if nc.partition_id_tensor else None
        )
        in_names, out_names, out_avals = [], [], []
        for alloc in nc.m.functions[0].allocations:
            if not isinstance(alloc, mybir.MemoryLocationSet):
                continue
            name = alloc.memorylocations[0].name
            if alloc.kind == "ExternalInput":
                if name != partition_name:
                    in_names.append(name)
            elif alloc.kind == "ExternalOutput":
                out_names.append(name)
                out_avals.append(
                    jax.core.ShapedArray(
                        tuple(alloc.tensor_shape), mybir.dt.np(alloc.dtype)
                    )
                )
        n_params = len(in_names)
        all_in_names = in_names + out_names
        if partition_name is not None:
            all_in_names = all_in_names + [partition_name]

        def _body(*args):
            operands = list(args)
            if partition_name is not None:
                operands.append(bass2jax.partition_id_tensor())
            outs = bass2jax._bass_exec_p.bind(
                *operands,
                out_avals=tuple(out_avals),
                in_names=tuple(all_in_names),
                out_names=tuple(out_names),
                lowering_input_output_aliases=(),
                sim_require_finite=True,
                sim_require_nnan=True,
                nc=nc,
            )
            return tuple(outs)

        devices = jax.devices()[:NCORES]
        mesh = Mesh(_np.asarray(devices), ("core",))
        n_outs = len(out_names)
        sharded = jax.jit(
            shard_map(
                _body,
                mesh=mesh,
                in_specs=(PartitionSpec("core"),) * (n_params + n_outs),
                out_specs=(PartitionSpec("core"),) * n_outs,
                check_rep=False,
            ),
            donate_argnums=tuple(range(n_params, n_params + n_outs)),
            keep_unused=True,
        )
        _JIT_CACHE[key] = (sharded, in_names, out_names, out_avals)

    sharded, in_names, out_names, out_avals = _JIT_CACHE[key]
    concat_in = [
        np.concatenate([np.asarray(m[name]) for m in in_maps], axis=0)
        for name in in_names
    ]
    concat_zeros = [
        np.zeros((NCORES * a.shape[0], *a.shape[1:]), a.dtype) for a in out_avals
    ]
    out_arrs = sharded(*concat_in, *concat_zeros)
    return [
        {
            name: np.asarray(out_arrs[i]).reshape(NCORES, *out_avals[i].shape)[c]
            for i, name in enumerate(out_names)
        }
        for c in range(NCORES)
    ]


def _run_spmd_safe(nc, in_maps):
    try:
        return _run_spmd(nc, in_maps)
    except Exception:
        from concourse.bass_utils import run_bass_kernel_spmd

        return run_bass_kernel_spmd(
            nc, in_maps, core_ids=list(range(NCORES))
        ).results


def kernel(queries, keys, values, attn_mask=None, **_unused):
    queries = np.asarray(queries)
    keys = np.asarray(keys)
    values = np.asarray(values)

    # ---- host prep: per-core sharded, time-last transposed q/k fp16 --------
    def _pack(x):
        # [B,N,L,H,E] -> [B,N,128,768] fp16, p-major (d = c*128+p, x = c*192+l)
        xt = x.transpose(0, 1, 3, 4, 2).reshape(B, N, 4, 128, L)
        return (
            np.ascontiguousarray(xt.transpose(0, 1, 3, 2, 4))
            .astype(np.float16)
            .reshape(B, N, 128, 768)
        )

    ktx = _pack(keys)
    qtx = _pack(queries)
    kqx = np.stack([ktx, qtx], axis=2)  # [B, N, 2, 128, 768]

    in_maps1 = []
    for i in range(NCORES):
        sl = slice(i * NLOC, (i + 1) * NLOC)
        in_maps1.append(
            {"kqx": np.ascontiguousarray(kqx[:, sl]).reshape(BN, 2, 128, 768)}
        )

    nc1 = _get_nc("corr")
    res1 = _run_spmd_safe(nc1, in_maps1)

    # ---- host: diag sums -> mean_value, top-k, softmax ---------------------
    # g5[core, quad, s, b4, u] fp16; c[tau] = sum_s G[s, (s+tau)%L]
    g_all = np.stack([r["g5"] for r in res1]).astype(np.float32)
    g2 = np.concatenate([g_all, g_all], axis=-1)  # [NC, 32, 192, 4, 384]
    st = g2.strides
    diag_view = np.lib.stride_tricks.as_strided(
        g2,
        shape=(NCORES, BN // 4, 4, L, L),  # [NC, quad, b4, s, tau]
        strides=(st[0], st[1], st[3], st[2] + st[4], st[4]),
    )
    c_all = diag_view.sum(axis=3, dtype=np.float64)  # [NC, quad, b4, tau]
    mean_value = (
        c_all.reshape(NCORES, B, NLOC, L).transpose(1, 0, 2, 3).reshape(B, N, L)
        / HE
    )
    z = mean_value.mean(axis=0)  # [N, L]
    # jax.lax.top_k semantics: descending, ties -> lowest index (stable)
    index = np.argsort(-z, axis=-1, kind="stable")[:, :TOPK]  # [N, K]
    w = np.take_along_axis(mean_value, index[None], axis=-1)  # [B, N, K]
    e = np.exp(w - w.max(axis=-1, keepdims=True))
    tmp_corr = e / e.sum(axis=-1, keepdims=True)  # [B, N, K]

    # ---- host: build A^T (shift matrices), shard v -------------------------
    # AT[b, n, t', tau] = w_j  where t' = (tau + d_j) % L
    pos = np.arange(L)
    rows = (pos[None, None, :] + index[:, :, None]) % L  # [N, K, L]
    AT = np.zeros((B, N, L, L), dtype=np.float32)
    bI = np.arange(B)[:, None, None, None]
    nI = np.arange(N)[None, :, None, None]
    AT[bI, nI, rows[None], pos[None, None, None, :]] = tmp_corr[:, :, :, None]
    AT = AT.astype(np.float16)

    v_flat = values.reshape(B, N, L, HD).astype(np.float16)

    in_maps2 = []
    for i in range(NCORES):
        sl = slice(i * NLOC, (i + 1) * NLOC)
        at_core = AT[:, sl].reshape(BN, L, L)
        # at6[quad, kc, p, b4, t]: t' = kc*96+p
        at_core = at_core.reshape(BN // 4, 4, 2, 96, L).transpose(0, 2, 3, 1, 4)
        v_core = v_flat[:, sl].reshape(BN // 4, 4, 2, 96, HD).transpose(
            0, 2, 3, 1, 4
        )
        in_maps2.append(
            {
                "at6": np.ascontiguousarray(at_core),
                "v6": np.ascontiguousarray(v_core),
            }
        )

    nc2 = _get_nc("agg")
    res2 = _run_spmd_safe(nc2, in_maps2)

    # o6[quad, c, p, b4, l] fp16: out element (4*quad+b4, hd=c*128+p, tau=l)
    o_all = np.stack([r["o6"] for r in res2])  # [NC, BN/4, 4, 128, 4, L]
    o_all = (
        o_all.astype(np.float32)
        .transpose(0, 1, 4, 2, 3, 5)  # [NC, quad, b4, c, p, l]
        .reshape(NCORES, BN, HD, L)
    )
    out = (
        o_all.transpose(0, 1, 3, 2)  # [NC, BN, L, HD]
        .reshape(NCORES, B, NLOC, L, H, D)
        .transpose(1, 0, 2, 3, 4, 5)
        .reshape(B, N, L, H, D)
    )
    return np.ascontiguousarray(out.astype(np.float32))


# revision 12
# speedup vs baseline: 1.0772x; 1.0772x over previous
"""Trainium2 Bass kernel for the AutoCorrelation module (Autoformer-style).

Shapes (hardcoded): B=8, N=128, L=192, H=8, E=64, D=64.

Math: for each (b, n):
  corr-mean  c[tau] = sum_s <Q_{(s+tau)%L}, K_s>  over the flattened (h,e) dim
             = circular-diagonal sums of the Gram matrix G[s,u] = <K_s, Q_u>
  top-5 delays per node from batch-averaged c (host), softmax weights (host),
  output o[tau, hd]  = sum_j w_j * v[(tau+d_j)%L, hd]
                     = (A @ V)[tau, hd]  with the sparse shift-matrix A (host-built)

Device work (8 cores, node axis sharded, 16 nodes/core, all 8 batches local):
  kernel 1: per-(b,n) Gram matrices, single-product fp16 with fp16 G store
            (measured on the real inputs: 1.8e-4 output rel err, zero top-5
            index flips vs the fp32 pipeline -- the hi/lo decomposition is
            not needed at the 2e-2 gate)
  kernel 2: per-(b,n) V^T-stationary shift-matrix matmul in fp16
Host work: transposes, diag-sums, top-k, softmax, A-matrix build, reassembly.
All DMA layouts keep >=1536B contiguous runs on BOTH sides of each descriptor
(the sub-512B runs were what held the old agg kernel at 249 GB/s).
"""

import numpy as np

import concourse.bass as bass  # noqa: F401
import concourse.mybir as mybir
import concourse.tile as tile
from concourse import bacc

B, N, L, H, E, D = 8, 128, 192, 8, 64, 64
HE = H * E            # 512
HD = H * D            # 512
NCORES = 8
NLOC = N // NCORES    # 16 nodes per core
BN = B * NLOC         # 128 (b, n) pairs per core
TOPK = 5              # int(log(192))

F32 = mybir.dt.float32
F16 = mybir.dt.float16


def _build_corr_nc(bn_count=BN, num_devices=NCORES, group=4):
    """Per (b,n): G[s,u] = sum_d k[s,d]*q[u,d], fp16 multiply / fp32 PSUM
    accumulate, G stored fp16.

    Input kqx[bn, p, t, x] fp16 (t: 0=k, 1=q; x = c*192 + l packs the d-chunk
    c (d = c*128 + p) and time l -> 3072B contiguous per (bn, p)).
    Output g5[quad, s, b4, u] fp16 (1536B runs per (quad, s)).
    bn's are processed in groups of 4 with one input DMA per group,
    alternating between the Sync and Scalar HWDGE rings.
    """
    nc = bacc.Bacc(
        "TRN2",
        target_bir_lowering=False,
        debug=False,
        enable_asserts=False,
        num_devices=num_devices,
    )
    kqx = nc.dram_tensor(
        "kqx", [bn_count, 128, 2, 768], F16, kind="ExternalInput"
    ).ap()
    g5 = nc.dram_tensor(
        "g5", [bn_count // 4, L, 4, L], F16, kind="ExternalOutput"
    ).ap()

    assert bn_count % group == 0
    with tile.TileContext(nc) as tc:
        with (
            tc.tile_pool(name="kin", bufs=4) as kpool,
            tc.tile_pool(name="gout", bufs=4) as gpool,
            tc.tile_pool(name="ps", bufs=2 * group, space="PSUM") as pspool,
        ):
            for qd in range(bn_count // group):
                g0 = qd * group
                kqtile = kpool.tile([128, group, 2, 768], F16)
                eng = nc.sync if qd % 2 == 0 else nc.scalar
                eng.dma_start(
                    out=kqtile[:],
                    in_=kqx[g0 : g0 + group].rearrange("b p t x -> p b t x"),
                )

                gtile = gpool.tile([128, group, 2 * L], F16)
                for i in range(group):
                    ps = pspool.tile([128, 2 * L], F32)
                    # m-chunks: G rows [0:128] -> ps[:, 0:L]; [128:192] -> ps[0:64, L:]
                    for msl, osl in (
                        (slice(0, 128), slice(0, L)),
                        (slice(128, 192), slice(L, 2 * L)),
                    ):
                        mlen = msl.stop - msl.start
                        for c in range(4):
                            x0 = c * L
                            nc.tensor.matmul(
                                ps[0:mlen, osl],
                                lhsT=kqtile[:, i, 0, x0 + msl.start : x0 + msl.stop],
                                rhs=kqtile[:, i, 1, x0 : x0 + L],
                                start=(c == 0),
                                stop=(c == 3),
                            )
                    nc.vector.tensor_copy(gtile[0:128, i, 0:L], ps[0:128, 0:L])
                    nc.scalar.copy(gtile[0:64, i, L : 2 * L], ps[0:64, L : 2 * L])

                # outputs on the SWDGE (gpsimd) ring; inputs own both HWDGE rings
                nc.gpsimd.dma_start(out=g5[qd, 0:128], in_=gtile[0:128, :, 0:L])
                nc.gpsimd.dma_start(
                    out=g5[qd, 128:192], in_=gtile[0:64, :, L : 2 * L]
                )

    nc.compile()
    return nc


def _build_agg_nc(bn_count=BN, num_devices=NCORES):
    """Per (b,n): o[hd, tau] = sum_t' v[t', hd] * at[t', tau], fp16 in/out.

    V is the stationary operand (full hd-chunk columns), AT the moving one;
    output is hd-major and transposed back on the host. Tile free-dim order
    is (kc, b4, .) so every DMA descriptor is a 1536B/4KB contiguous run on
    both the DRAM and SBUF side.
    """
    nc = bacc.Bacc(
        "TRN2",
        target_bir_lowering=False,
        debug=False,
        enable_asserts=False,
        num_devices=num_devices,
    )
    assert bn_count % 4 == 0
    nquad = bn_count // 4
    # at6[quad, p, kc, b4, t]: t' = kc*96 + p -> 3072B runs per (quad, p)
    at6 = nc.dram_tensor(
        "at6", [nquad, 96, 2, 4, L], F16, kind="ExternalInput"
    ).ap()
    # v6[quad, p, kc, b4, d] -> 8KB runs per (quad, p)
    v6 = nc.dram_tensor(
        "v6", [nquad, 96, 2, 4, HD], F16, kind="ExternalInput"
    ).ap()
    # o7[quad, p, half, cc, b4, l]: output element (bn, hd=(half*2+cc)*128+p,
    # tau=l) -> 6KB runs per (quad, p)
    o7 = nc.dram_tensor(
        "o7", [nquad, 128, 2, 2, 4, L], F16, kind="ExternalOutput"
    ).ap()

    with tile.TileContext(nc) as tc:
        with (
            tc.tile_pool(name="ain", bufs=4) as apool,
            tc.tile_pool(name="vin", bufs=4) as vpool,
            tc.tile_pool(name="oout", bufs=4) as opool,
            tc.tile_pool(name="ps", bufs=8, space="PSUM") as pspool,
        ):
            for qd in range(nquad):
                # alternate input rings so each HWDGE ring carries ~half
                ea, ev = (nc.scalar, nc.sync) if qd % 2 == 0 else (nc.sync, nc.scalar)
                atile = apool.tile([96, 2, 4, L], F16)
                ea.dma_start(out=atile[:], in_=at6[qd])
                vtile = vpool.tile([96, 2, 4, HD], F16)
                ev.dma_start(out=vtile[:], in_=v6[qd])

                # otile free layout: (half, cc, b4, l); hd-chunk c = half*2+cc
                otile = opool.tile([128, 2, 2, 4, L], F16)
                for i in range(4):
                    pss = [
                        pspool.tile([128, 2 * L], F32, name="ps", tag="ps")
                        for _ in range(2)
                    ]
                    for c in range(4):
                        ps = pss[c // 2][0:128, (c % 2) * L : (c % 2 + 1) * L]
                        for kc in range(2):
                            nc.tensor.matmul(
                                ps,
                                lhsT=vtile[:, kc, i, c * 128 : (c + 1) * 128],
                                rhs=atile[:, kc, i, :],
                                start=(kc == 0),
                                stop=(kc == 1),
                            )
                    nc.vector.tensor_copy(
                        otile[:, 0, :, i, :],
                        pss[0][:].rearrange("p (cc l) -> p cc l", cc=2),
                    )
                    nc.scalar.copy(
                        otile[:, 1, :, i, :],
                        pss[1][:].rearrange("p (cc l) -> p cc l", cc=2),
                    )

                nc.gpsimd.dma_start(out=o7[qd], in_=otile[:])

    nc.compile()
    return nc


_NC_CACHE = {}


def _get_nc(name):
    if name not in _NC_CACHE:
        _NC_CACHE[name] = {"corr": _build_corr_nc, "agg": _build_agg_nc}[name]()
    return _NC_CACHE[name]


_JIT_CACHE = {}


def _run_spmd(nc, in_maps):
    """run_bass_kernel_spmd's axon path with the jitted executable cached
    per-module, so repeat kernel() calls don't re-trace/re-compile."""
    import jax
    import numpy as _np
    from jax.experimental.shard_map import shard_map
    from jax.sharding import Mesh, PartitionSpec

    from concourse import bass2jax

    key = id(nc)
    if key not in _JIT_CACHE:
        bass2jax.install_neuronx_cc_hook()
        partition_name = (
            nc.partition_id_tensor.name if nc.partition_id_tensor else None
        )
        in_names, out_names, out_avals = [], [], []
        for alloc in nc.m.functions[0].allocations:
            if not isinstance(alloc, mybir.MemoryLocationSet):
                continue
            name = alloc.memorylocations[0].name
            if alloc.kind == "ExternalInput":
                if name != partition_name:
                    in_names.append(name)
            elif alloc.kind == "ExternalOutput":
                out_names.append(name)
                out_avals.append(
                    jax.core.ShapedArray(
                        tuple(alloc.tensor_shape), mybir.dt.np(alloc.dtype)
                    )
                )
        n_params = len(in_names)
        all_in_names = in_names + out_names
        if partition_name is not None:
            all_in_names = all_in_names + [partition_name]

        def _body(*args):
            operands = list(args)
            if partition_name is not None:
                operands.append(bass2jax.partition_id_tensor())
            outs = bass2jax._bass_exec_p.bind(
                *operands,
                out_avals=tuple(out_avals),
                in_names=tuple(all_in_names),
                out_names=tuple(out_names),
                lowering_input_output_aliases=(),
                sim_require_finite=True,
                sim_require_nnan=True,
                nc=nc,
            )
            return tuple(outs)

        devices = jax.devices()[:NCORES]
        mesh = Mesh(_np.asarray(devices), ("core",))
        n_outs = len(out_names)
        sharded = jax.jit(
            shard_map(
                _body,
                mesh=mesh,
                in_specs=(PartitionSpec("core"),) * (n_params + n_outs),
                out_specs=(PartitionSpec("core"),) * n_outs,
                check_rep=False,
            ),
            donate_argnums=tuple(range(n_params, n_params + n_outs)),
            keep_unused=True,
        )
        _JIT_CACHE[key] = (sharded, in_names, out_names, out_avals)

    sharded, in_names, out_names, out_avals = _JIT_CACHE[key]
    concat_in = [
        np.concatenate([np.asarray(m[name]) for m in in_maps], axis=0)
        for name in in_names
    ]
    concat_zeros = [
        np.zeros((NCORES * a.shape[0], *a.shape[1:]), a.dtype) for a in out_avals
    ]
    out_arrs = sharded(*concat_in, *concat_zeros)
    return [
        {
            name: np.asarray(out_arrs[i]).reshape(NCORES, *out_avals[i].shape)[c]
            for i, name in enumerate(out_names)
        }
        for c in range(NCORES)
    ]


def _run_spmd_safe(nc, in_maps):
    try:
        return _run_spmd(nc, in_maps)
    except Exception:
        from concourse.bass_utils import run_bass_kernel_spmd

        return run_bass_kernel_spmd(
            nc, in_maps, core_ids=list(range(NCORES))
        ).results


def kernel(queries, keys, values, attn_mask=None, **_unused):
    queries = np.asarray(queries)
    keys = np.asarray(keys)
    values = np.asarray(values)

    # ---- host prep: per-core sharded, time-last transposed q/k fp16 --------
    def _pack(x):
        # [B,N,L,H,E] -> [B,N,128,768] fp16, p-major (d = c*128+p, x = c*192+l)
        xt = x.transpose(0, 1, 3, 4, 2).reshape(B, N, 4, 128, L)
        return (
            np.ascontiguousarray(xt.transpose(0, 1, 3, 2, 4))
            .astype(np.float16)
            .reshape(B, N, 128, 768)
        )

    ktx = _pack(keys)
    qtx = _pack(queries)
    kqx = np.stack([ktx, qtx], axis=3)  # [B, N, 128, 2, 768]

    in_maps1 = []
    for i in range(NCORES):
        sl = slice(i * NLOC, (i + 1) * NLOC)
        in_maps1.append(
            {"kqx": np.ascontiguousarray(kqx[:, sl]).reshape(BN, 128, 2, 768)}
        )

    nc1 = _get_nc("corr")
    res1 = _run_spmd_safe(nc1, in_maps1)

    # ---- host: diag sums -> mean_value, top-k, softmax ---------------------
    # g5[core, quad, s, b4, u] fp16; c[tau] = sum_s G[s, (s+tau)%L]
    g_all = np.stack([r["g5"] for r in res1]).astype(np.float32)
    g2 = np.concatenate([g_all, g_all], axis=-1)  # [NC, 32, 192, 4, 384]
    st = g2.strides
    diag_view = np.lib.stride_tricks.as_strided(
        g2,
        shape=(NCORES, BN // 4, 4, L, L),  # [NC, quad, b4, s, tau]
        strides=(st[0], st[1], st[3], st[2] + st[4], st[4]),
    )
    c_all = diag_view.sum(axis=3, dtype=np.float64)  # [NC, quad, b4, tau]
    mean_value = (
        c_all.reshape(NCORES, B, NLOC, L).transpose(1, 0, 2, 3).reshape(B, N, L)
        / HE
    )
    z = mean_value.mean(axis=0)  # [N, L]
    # jax.lax.top_k semantics: descending, ties -> lowest index (stable)
    index = np.argsort(-z, axis=-1, kind="stable")[:, :TOPK]  # [N, K]
    w = np.take_along_axis(mean_value, index[None], axis=-1)  # [B, N, K]
    e = np.exp(w - w.max(axis=-1, keepdims=True))
    tmp_corr = e / e.sum(axis=-1, keepdims=True)  # [B, N, K]

    # ---- host: build A^T (shift matrices), shard v -------------------------
    # AT[b, n, t', tau] = w_j  where t' = (tau + d_j) % L
    pos = np.arange(L)
    rows = (pos[None, None, :] + index[:, :, None]) % L  # [N, K, L]
    AT = np.zeros((B, N, L, L), dtype=np.float32)
    bI = np.arange(B)[:, None, None, None]
    nI = np.arange(N)[None, :, None, None]
    AT[bI, nI, rows[None], pos[None, None, None, :]] = tmp_corr[:, :, :, None]
    AT = AT.astype(np.float16)

    v_flat = values.reshape(B, N, L, HD).astype(np.float16)

    in_maps2 = []
    for i in range(NCORES):
        sl = slice(i * NLOC, (i + 1) * NLOC)
        at_core = AT[:, sl].reshape(BN, L, L)
        # at6[quad, p, kc, b4, t]: t' = kc*96+p
        at_core = at_core.reshape(BN // 4, 4, 2, 96, L).transpose(0, 3, 2, 1, 4)
        v_core = v_flat[:, sl].reshape(BN // 4, 4, 2, 96, HD).transpose(
            0, 3, 2, 1, 4
        )
        in_maps2.append(
            {
                "at6": np.ascontiguousarray(at_core),
                "v6": np.ascontiguousarray(v_core),
            }
        )

    nc2 = _get_nc("agg")
    res2 = _run_spmd_safe(nc2, in_maps2)

    # o7[quad, p, half, cc, b4, l] fp16: out element
    # (4*quad+b4, hd=(half*2+cc)*128+p, tau=l)
    o_all = np.stack([r["o7"] for r in res2])  # [NC, BN/4, 128, 2, 2, 4, L]
    o_all = (
        o_all.astype(np.float32)
        .transpose(0, 1, 5, 3, 4, 2, 6)  # [NC, quad, b4, half, cc, p, l]
        .reshape(NCORES, BN, HD, L)
    )
    out = (
        o_all.transpose(0, 1, 3, 2)  # [NC, BN, L, HD]
        .reshape(NCORES, B, NLOC, L, H, D)
        .transpose(1, 0, 2, 3, 4, 5)
        .reshape(B, N, L, H, D)
    )
    return np.ascontiguousarray(out.astype(np.float32))


# revision 15
# speedup vs baseline: 1.0842x; 1.0065x over previous
"""Trainium2 Bass kernel for the AutoCorrelation module (Autoformer-style).

Shapes (hardcoded): B=8, N=128, L=192, H=8, E=64, D=64.

Math: for each (b, n):
  corr-mean  c[tau] = sum_s <Q_{(s+tau)%L}, K_s>  over the flattened (h,e) dim
             = circular-diagonal sums of the Gram matrix G[s,u] = <K_s, Q_u>
  top-5 delays per node from batch-averaged c (host), softmax weights (host),
  output o[tau, hd]  = sum_j w_j * v[(tau+d_j)%L, hd]
                     = (A @ V)[tau, hd]  with the sparse shift-matrix A (host-built)

Device work (8 cores, node axis sharded, 16 nodes/core, all 8 batches local):
  kernel 1: per-(b,n) Gram matrices, single-product fp16 with fp16 G store
            (measured on the real inputs: 1.8e-4 output rel err, zero top-5
            index flips vs the fp32 pipeline -- the hi/lo decomposition is
            not needed at the 2e-2 gate)
  kernel 2: per-(b,n) V^T-stationary shift-matrix matmul in fp16
Host work: transposes, diag-sums, top-k, softmax, A-matrix build, reassembly.
All DMA layouts keep >=1536B contiguous runs on BOTH sides of each descriptor
(the sub-512B runs were what held the old agg kernel at 249 GB/s).
"""

import numpy as np

import concourse.bass as bass  # noqa: F401
import concourse.mybir as mybir
import concourse.tile as tile
from concourse import bacc

B, N, L, H, E, D = 8, 128, 192, 8, 64, 64
HE = H * E            # 512
HD = H * D            # 512
NCORES = 8
NLOC = N // NCORES    # 16 nodes per core
BN = B * NLOC         # 128 (b, n) pairs per core
TOPK = 5              # int(log(192))

F32 = mybir.dt.float32
F16 = mybir.dt.float16


def _build_corr_nc(bn_count=BN, num_devices=NCORES, group=4):
    """Per (b,n): G[s,u] = sum_d k[s,d]*q[u,d], fp16 multiply / fp32 PSUM
    accumulate, G stored fp16.

    Input kqx[bn, p, t, x] fp16 (t: 0=k, 1=q; x = c*192 + l packs the d-chunk
    c (d = c*128 + p) and time l -> 3072B contiguous per (bn, p)).
    Output g5[quad, s, b4, u] fp16 (1536B runs per (quad, s)).
    bn's are processed in groups of 4 with one input DMA per group,
    alternating between the Sync and Scalar HWDGE rings.
    """
    nc = bacc.Bacc(
        "TRN2",
        target_bir_lowering=False,
        debug=False,
        enable_asserts=False,
        num_devices=num_devices,
    )
    kqx = nc.dram_tensor(
        "kqx", [bn_count, 128, 2, 768], F16, kind="ExternalInput"
    ).ap()
    g5 = nc.dram_tensor(
        "g5", [bn_count // 4, L, 4, L], F16, kind="ExternalOutput"
    ).ap()

    assert bn_count % group == 0
    with tile.TileContext(nc) as tc:
        with (
            tc.tile_pool(name="kin", bufs=6) as kpool,
            tc.tile_pool(name="gout", bufs=6) as gpool,
            tc.tile_pool(name="ps", bufs=2 * group, space="PSUM") as pspool,
        ):
            for qd in range(bn_count // group):
                g0 = qd * group
                kqtile = kpool.tile([128, group, 2, 768], F16)
                eng = nc.sync if qd % 2 == 0 else nc.scalar
                eng.dma_start(
                    out=kqtile[:],
                    in_=kqx[g0 : g0 + group].rearrange("b p t x -> p b t x"),
                )

                gtile = gpool.tile([128, group, 2 * L], F16)
                for i in range(group):
                    ps = pspool.tile([128, 2 * L], F32)
                    # m-chunks: G rows [0:128] -> ps[:, 0:L]; [128:192] -> ps[0:64, L:]
                    for msl, osl in (
                        (slice(0, 128), slice(0, L)),
                        (slice(128, 192), slice(L, 2 * L)),
                    ):
                        mlen = msl.stop - msl.start
                        for c in range(4):
                            x0 = c * L
                            nc.tensor.matmul(
                                ps[0:mlen, osl],
                                lhsT=kqtile[:, i, 0, x0 + msl.start : x0 + msl.stop],
                                rhs=kqtile[:, i, 1, x0 : x0 + L],
                                start=(c == 0),
                                stop=(c == 3),
                            )
                    # one full-width cast; rows 64:128 of the second half are
                    # never read by the output DMA (G rows 128:192 live in
                    # ps[0:64, L:2L]) so copying them is harmless. Keeps the
                    # scalar engine free to issue its ring's input DMAs.
                    nc.vector.tensor_copy(gtile[:, i, :], ps[:])

                # outputs on the SWDGE (gpsimd) ring; inputs own both HWDGE rings
                nc.gpsimd.dma_start(out=g5[qd, 0:128], in_=gtile[0:128, :, 0:L])
                nc.gpsimd.dma_start(
                    out=g5[qd, 128:192], in_=gtile[0:64, :, L : 2 * L]
                )

    nc.compile()
    return nc


def _build_agg_nc(bn_count=BN, num_devices=NCORES):
    """Per (b,n): o[hd, tau] = sum_t' v[t', hd] * at[t', tau], fp16 in/out.

    V is the stationary operand (full hd-chunk columns), AT the moving one;
    output is hd-major and transposed back on the host. Tile free-dim order
    is (kc, b4, .) so every DMA descriptor is a 1536B/4KB contiguous run on
    both the DRAM and SBUF side.
    """
    nc = bacc.Bacc(
        "TRN2",
        target_bir_lowering=False,
        debug=False,
        enable_asserts=False,
        num_devices=num_devices,
    )
    assert bn_count % 4 == 0
    nquad = bn_count // 4
    # at6[quad, p, kc, b4, t]: t' = kc*96 + p -> 3072B runs per (quad, p)
    at6 = nc.dram_tensor(
        "at6", [nquad, 96, 2, 4, L], F16, kind="ExternalInput"
    ).ap()
    # v6[quad, p, kc, b4, d] -> 8KB runs per (quad, p)
    v6 = nc.dram_tensor(
        "v6", [nquad, 96, 2, 4, HD], F16, kind="ExternalInput"
    ).ap()
    # o7[quad, p, half, cc, b4, l]: output element (bn, hd=(half*2+cc)*128+p,
    # tau=l) -> 6KB runs per (quad, p)
    o7 = nc.dram_tensor(
        "o7", [nquad, 128, 2, 2, 4, L], F16, kind="ExternalOutput"
    ).ap()

    with tile.TileContext(nc) as tc:
        with (
            tc.tile_pool(name="ain", bufs=6) as apool,
            tc.tile_pool(name="vin", bufs=6) as vpool,
            tc.tile_pool(name="oout", bufs=6) as opool,
            tc.tile_pool(name="ps", bufs=8, space="PSUM") as pspool,
        ):
            for qd in range(nquad):
                # both inputs on the Sync ring: the Scalar engine then has no
                # dma_start sequencer-waits interleaved with its copy work
                atile = apool.tile([96, 2, 4, L], F16)
                nc.sync.dma_start(out=atile[:], in_=at6[qd])
                vtile = vpool.tile([96, 2, 4, HD], F16)
                nc.sync.dma_start(out=vtile[:], in_=v6[qd])

                # otile free layout: (half, cc, b4, l); hd-chunk c = half*2+cc
                otile = opool.tile([128, 2, 2, 4, L], F16)
                for i in range(4):
                    pss = [
                        pspool.tile([128, 2 * L], F32, name="ps", tag="ps")
                        for _ in range(2)
                    ]
                    for c in range(4):
                        ps = pss[c // 2][0:128, (c % 2) * L : (c % 2 + 1) * L]
                        for kc in range(2):
                            nc.tensor.matmul(
                                ps,
                                lhsT=vtile[:, kc, i, c * 128 : (c + 1) * 128],
                                rhs=atile[:, kc, i, :],
                                start=(kc == 0),
                                stop=(kc == 1),
                            )
                    nc.vector.tensor_copy(
                        otile[:, 0, :, i, :],
                        pss[0][:].rearrange("p (cc l) -> p cc l", cc=2),
                    )
                    nc.scalar.copy(
                        otile[:, 1, :, i, :],
                        pss[1][:].rearrange("p (cc l) -> p cc l", cc=2),
                    )

                nc.gpsimd.dma_start(out=o7[qd], in_=otile[:])

    nc.compile()
    return nc


_NC_CACHE = {}


def _get_nc(name):
    if name not in _NC_CACHE:
        _NC_CACHE[name] = {"corr": _build_corr_nc, "agg": _build_agg_nc}[name]()
    return _NC_CACHE[name]


_JIT_CACHE = {}


def _run_spmd(nc, in_maps):
    """run_bass_kernel_spmd's axon path with the jitted executable cached
    per-module, so repeat kernel() calls don't re-trace/re-compile."""
    import jax
    import numpy as _np
    from jax.experimental.shard_map import shard_map
    from jax.sharding import Mesh, PartitionSpec

    from concourse import bass2jax

    key = id(nc)
    if key not in _JIT_CACHE:
        bass2jax.install_neuronx_cc_hook()
        partition_name = (
            nc.partition_id_tensor.name if nc.partition_id_tensor else None
        )
        in_names, out_names, out_avals = [], [], []
        for alloc in nc.m.functions[0].allocations:
            if not isinstance(alloc, mybir.MemoryLocationSet):
                continue
            name = alloc.memorylocations[0].name
            if alloc.kind == "ExternalInput":
                if name != partition_name:
                    in_names.append(name)
            elif alloc.kind == "ExternalOutput":
                out_names.append(name)
                out_avals.append(
                    jax.core.ShapedArray(
                        tuple(alloc.tensor_shape), mybir.dt.np(alloc.dtype)
                    )
                )
        n_params = len(in_names)
        all_in_names = in_names + out_names
        if partition_name is not None:
            all_in_names = all_in_names + [partition_name]

        def _body(*args):
            operands = list(args)
            if partition_name is not None:
                operands.append(bass2jax.partition_id_tensor())
            outs = bass2jax._bass_exec_p.bind(
                *operands,
                out_avals=tuple(out_avals),
                in_names=tuple(all_in_names),
                out_names=tuple(out_names),
                lowering_input_output_aliases=(),
                sim_require_finite=True,
                sim_require_nnan=True,
                nc=nc,
            )
            return tuple(outs)

        devices = jax.devices()[:NCORES]
        mesh = Mesh(_np.asarray(devices), ("core",))
        n_outs = len(out_names)
        sharded = jax.jit(
            shard_map(
                _body,
                mesh=mesh,
                in_specs=(PartitionSpec("core"),) * (n_params + n_outs),
                out_specs=(PartitionSpec("core"),) * n_outs,
                check_rep=False,
            ),
            donate_argnums=tuple(range(n_params, n_params + n_outs)),
            keep_unused=True,
        )
        _JIT_CACHE[key] = (sharded, in_names, out_names, out_avals)

    sharded, in_names, out_names, out_avals = _JIT_CACHE[key]
    concat_in = [
        np.concatenate([np.asarray(m[name]) for m in in_maps], axis=0)
        for name in in_names
    ]
    concat_zeros = [
        np.zeros((NCORES * a.shape[0], *a.shape[1:]), a.dtype) for a in out_avals
    ]
    out_arrs = sharded(*concat_in, *concat_zeros)
    return [
        {
            name: np.asarray(out_arrs[i]).reshape(NCORES, *out_avals[i].shape)[c]
            for i, name in enumerate(out_names)
        }
        for c in range(NCORES)
    ]


def _run_spmd_safe(nc, in_maps):
    try:
        return _run_spmd(nc, in_maps)
    except Exception:
        from concourse.bass_utils import run_bass_kernel_spmd

        return run_bass_kernel_spmd(
            nc, in_maps, core_ids=list(range(NCORES))
        ).results


def kernel(queries, keys, values, attn_mask=None, **_unused):
    queries = np.asarray(queries)
    keys = np.asarray(keys)
    values = np.asarray(values)

    # ---- host prep: per-core sharded, time-last transposed q/k fp16 --------
    def _pack(x):
        # [B,N,L,H,E] -> [B,N,128,768] fp16, p-major (d = c*128+p, x = c*192+l)
        xt = x.transpose(0, 1, 3, 4, 2).reshape(B, N, 4, 128, L)
        return (
            np.ascontiguousarray(xt.transpose(0, 1, 3, 2, 4))
            .astype(np.float16)
            .reshape(B, N, 128, 768)
        )

    ktx = _pack(keys)
    qtx = _pack(queries)
    kqx = np.stack([ktx, qtx], axis=3)  # [B, N, 128, 2, 768]

    in_maps1 = []
    for i in range(NCORES):
        sl = slice(i * NLOC, (i + 1) * NLOC)
        in_maps1.append(
            {"kqx": np.ascontiguousarray(kqx[:, sl]).reshape(BN, 128, 2, 768)}
        )

    nc1 = _get_nc("corr")
    res1 = _run_spmd_safe(nc1, in_maps1)

    # ---- host: diag sums -> mean_value, top-k, softmax ---------------------
    # g5[core, quad, s, b4, u] fp16; c[tau] = sum_s G[s, (s+tau)%L]
    g_all = np.stack([r["g5"] for r in res1]).astype(np.float32)
    g2 = np.concatenate([g_all, g_all], axis=-1)  # [NC, 32, 192, 4, 384]
    st = g2.strides
    diag_view = np.lib.stride_tricks.as_strided(
        g2,
        shape=(NCORES, BN // 4, 4, L, L),  # [NC, quad, b4, s, tau]
        strides=(st[0], st[1], st[3], st[2] + st[4], st[4]),
    )
    c_all = diag_view.sum(axis=3, dtype=np.float64)  # [NC, quad, b4, tau]
    mean_value = (
        c_all.reshape(NCORES, B, NLOC, L).transpose(1, 0, 2, 3).reshape(B, N, L)
        / HE
    )
    z = mean_value.mean(axis=0)  # [N, L]
    # jax.lax.top_k semantics: descending, ties -> lowest index (stable)
    index = np.argsort(-z, axis=-1, kind="stable")[:, :TOPK]  # [N, K]
    w = np.take_along_axis(mean_value, index[None], axis=-1)  # [B, N, K]
    e = np.exp(w - w.max(axis=-1, keepdims=True))
    tmp_corr = e / e.sum(axis=-1, keepdims=True)  # [B, N, K]

    # ---- host: build A^T (shift matrices), shard v -------------------------
    # AT[b, n, t', tau] = w_j  where t' = (tau + d_j) % L
    pos = np.arange(L)
    rows = (pos[None, None, :] + index[:, :, None]) % L  # [N, K, L]
    AT = np.zeros((B, N, L, L), dtype=np.float32)
    bI = np.arange(B)[:, None, None, None]
    nI = np.arange(N)[None, :, None, None]
    AT[bI, nI, rows[None], pos[None, None, None, :]] = tmp_corr[:, :, :, None]
    AT = AT.astype(np.float16)

    v_flat = values.reshape(B, N, L, HD).astype(np.float16)

    in_maps2 = []
    for i in range(NCORES):
        sl = slice(i * NLOC, (i + 1) * NLOC)
        at_core = AT[:, sl].reshape(BN, L, L)
        # at6[quad, p, kc, b4, t]: t' = kc*96+p
        at_core = at_core.reshape(BN // 4, 4, 2, 96, L).transpose(0, 3, 2, 1, 4)
        v_core = v_flat[:, sl].reshape(BN // 4, 4, 2, 96, HD).transpose(
            0, 3, 2, 1, 4
        )
        in_maps2.append(
            {
                "at6": np.ascontiguousarray(at_core),
                "v6": np.ascontiguousarray(v_core),
            }
        )

    nc2 = _get_nc("agg")
    res2 = _run_spmd_safe(nc2, in_maps2)

    # o7[quad, p, half, cc, b4, l] fp16: out element
    # (4*quad+b4, hd=(half*2+cc)*128+p, tau=l)
    o_all = np.stack([r["o7"] for r in res2])  # [NC, BN/4, 128, 2, 2, 4, L]
    o_all = (
        o_all.astype(np.float32)
        .transpose(0, 1, 5, 3, 4, 2, 6)  # [NC, quad, b4, half, cc, p, l]
        .reshape(NCORES, BN, HD, L)
    )
    out = (
        o_all.transpose(0, 1, 3, 2)  # [NC, BN, L, HD]
        .reshape(NCORES, B, NLOC, L, H, D)
        .transpose(1, 0, 2, 3, 4, 5)
        .reshape(B, N, L, H, D)
    )
    return np.ascontiguousarray(out.astype(np.float32))


# revision 16
# speedup vs baseline: 1.1060x; 1.0201x over previous
"""Trainium2 Bass kernel for the AutoCorrelation module (Autoformer-style).

Shapes (hardcoded): B=8, N=128, L=192, H=8, E=64, D=64.

Math: for each (b, n):
  corr-mean  c[tau] = sum_s <Q_{(s+tau)%L}, K_s>  over the flattened (h,e) dim
             = circular-diagonal sums of the Gram matrix G[s,u] = <K_s, Q_u>
  top-5 delays per node from batch-averaged c (host), softmax weights (host),
  output o[tau, hd]  = sum_j w_j * v[(tau+d_j)%L, hd]
                     = (A @ V)[tau, hd]  with the sparse shift-matrix A (host-built)

Device work (8 cores, node axis sharded, 16 nodes/core, all 8 batches local):
  kernel 1: per-(b,n) Gram matrices, single-product fp16 with fp16 G store
            (measured on the real inputs: 1.8e-4 output rel err, zero top-5
            index flips vs the fp32 pipeline -- the hi/lo decomposition is
            not needed at the 2e-2 gate)
  kernel 2: per-(b,n) V^T-stationary shift-matrix matmul in fp16
Host work: transposes, diag-sums, top-k, softmax, A-matrix build, reassembly.
All DMA layouts keep >=1536B contiguous runs on BOTH sides of each descriptor
(the sub-512B runs were what held the old agg kernel at 249 GB/s).
"""

import numpy as np

import concourse.bass as bass  # noqa: F401
import concourse.mybir as mybir
import concourse.tile as tile
from concourse import bacc

B, N, L, H, E, D = 8, 128, 192, 8, 64, 64
HE = H * E            # 512
HD = H * D            # 512
NCORES = 8
NLOC = N // NCORES    # 16 nodes per core
BN = B * NLOC         # 128 (b, n) pairs per core
TOPK = 5              # int(log(192))

F32 = mybir.dt.float32
F16 = mybir.dt.float16


def _build_corr_nc(bn_count=BN, num_devices=NCORES, group=4):
    """Per (b,n): G[s,u] = sum_d k[s,d]*q[u,d], fp16 multiply / fp32 PSUM
    accumulate, G stored fp16.

    Input kqx[bn, p, t, x] fp16 (t: 0=k, 1=q; x = c*192 + l packs the d-chunk
    c (d = c*128 + p) and time l -> 3072B contiguous per (bn, p)).
    Output g5[quad, s, b4, u] fp16 (1536B runs per (quad, s)).
    bn's are processed in groups of 4 with one input DMA per group,
    alternating between the Sync and Scalar HWDGE rings.
    """
    nc = bacc.Bacc(
        "TRN2",
        target_bir_lowering=False,
        debug=False,
        enable_asserts=False,
        num_devices=num_devices,
    )
    kqx = nc.dram_tensor(
        "kqx", [bn_count, 128, 2, 768], F16, kind="ExternalInput"
    ).ap()
    g5 = nc.dram_tensor(
        "g5", [bn_count // 4, L, 4, L], F16, kind="ExternalOutput"
    ).ap()

    assert bn_count % group == 0
    with tile.TileContext(nc) as tc:
        with (
            tc.tile_pool(name="kin", bufs=6) as kpool,
            tc.tile_pool(name="gout", bufs=6) as gpool,
            tc.tile_pool(name="ps", bufs=2 * group, space="PSUM") as pspool,
        ):
            for qd in range(bn_count // group):
                g0 = qd * group
                kqtile = kpool.tile([128, group, 2, 768], F16)
                eng = nc.sync if qd % 2 == 0 else nc.scalar
                eng.dma_start(
                    out=kqtile[:],
                    in_=kqx[g0 : g0 + group].rearrange("b p t x -> p b t x"),
                )

                gtile = gpool.tile([128, group, 2 * L], F16)
                for i in range(group):
                    ps = pspool.tile([128, 2 * L], F32)
                    # m-chunks: G rows [0:128] -> ps[:, 0:L]; [128:192] -> ps[0:64, L:]
                    for msl, osl in (
                        (slice(0, 128), slice(0, L)),
                        (slice(128, 192), slice(L, 2 * L)),
                    ):
                        mlen = msl.stop - msl.start
                        for c in range(4):
                            x0 = c * L
                            nc.tensor.matmul(
                                ps[0:mlen, osl],
                                lhsT=kqtile[:, i, 0, x0 + msl.start : x0 + msl.stop],
                                rhs=kqtile[:, i, 1, x0 : x0 + L],
                                start=(c == 0),
                                stop=(c == 3),
                            )
                    # one full-width cast; rows 64:128 of the second half are
                    # never read by the output DMA (G rows 128:192 live in
                    # ps[0:64, L:2L]) so copying them is harmless. Keeps the
                    # scalar engine free to issue its ring's input DMAs.
                    nc.vector.tensor_copy(gtile[:, i, :], ps[:])

                # outputs on the SWDGE (gpsimd) ring; inputs own both HWDGE rings
                nc.gpsimd.dma_start(out=g5[qd, 0:128], in_=gtile[0:128, :, 0:L])
                nc.gpsimd.dma_start(
                    out=g5[qd, 128:192], in_=gtile[0:64, :, L : 2 * L]
                )

    nc.compile()
    return nc


def _build_agg_nc(bn_count=BN, num_devices=NCORES):
    """Per (b,n): o[hd, tau] = sum_t' v[t', hd] * at[t', tau], fp16 in/out.

    V is the stationary operand (full hd-chunk columns), AT the moving one;
    output is hd-major and transposed back on the host. Tile free-dim order
    is (kc, b4, .) so every DMA descriptor is a 1536B/4KB contiguous run on
    both the DRAM and SBUF side.
    """
    nc = bacc.Bacc(
        "TRN2",
        target_bir_lowering=False,
        debug=False,
        enable_asserts=False,
        num_devices=num_devices,
    )
    assert bn_count % 4 == 0
    nquad = bn_count // 4
    # at6[quad, p, kc, b4, t]: t' = kc*96 + p -> 3072B runs per (quad, p)
    at6 = nc.dram_tensor(
        "at6", [nquad, 96, 2, 4, L], F16, kind="ExternalInput"
    ).ap()
    # v6[quad, p, kc, b4, d] -> 8KB runs per (quad, p)
    v6 = nc.dram_tensor(
        "v6", [nquad, 96, 2, 4, HD], F16, kind="ExternalInput"
    ).ap()
    # o7[quad, p, half, cc, b4, l]: output element (bn, hd=(half*2+cc)*128+p,
    # tau=l) -> 6KB runs per (quad, p)
    o7 = nc.dram_tensor(
        "o7", [nquad, 128, 2, 2, 4, L], F16, kind="ExternalOutput"
    ).ap()

    with tile.TileContext(nc) as tc:
        with (
            tc.tile_pool(name="ain", bufs=6) as apool,
            tc.tile_pool(name="vin", bufs=6) as vpool,
            tc.tile_pool(name="oout", bufs=6) as opool,
            tc.tile_pool(name="ps", bufs=8, space="PSUM") as pspool,
        ):
            for qd in range(nquad):
                # alternate input rings so each HWDGE ring carries ~half
                ea, ev = (nc.scalar, nc.sync) if qd % 2 == 0 else (nc.sync, nc.scalar)
                atile = apool.tile([96, 2, 4, L], F16)
                ea.dma_start(out=atile[:], in_=at6[qd])
                vtile = vpool.tile([96, 2, 4, HD], F16)
                ev.dma_start(out=vtile[:], in_=v6[qd])

                # otile free layout: (half, cc, b4, l); hd-chunk c = half*2+cc
                otile = opool.tile([128, 2, 2, 4, L], F16)
                for i in range(4):
                    pss = [
                        pspool.tile([128, 2 * L], F32, name="ps", tag="ps")
                        for _ in range(2)
                    ]
                    for c in range(4):
                        ps = pss[c // 2][0:128, (c % 2) * L : (c % 2 + 1) * L]
                        for kc in range(2):
                            nc.tensor.matmul(
                                ps,
                                lhsT=vtile[:, kc, i, c * 128 : (c + 1) * 128],
                                rhs=atile[:, kc, i, :],
                                start=(kc == 0),
                                stop=(kc == 1),
                            )
                    nc.vector.tensor_copy(
                        otile[:, 0, :, i, :],
                        pss[0][:].rearrange("p (cc l) -> p cc l", cc=2),
                    )
                    nc.scalar.copy(
                        otile[:, 1, :, i, :],
                        pss[1][:].rearrange("p (cc l) -> p cc l", cc=2),
                    )

                nc.gpsimd.dma_start(out=o7[qd], in_=otile[:])

    nc.compile()
    return nc


_NC_CACHE = {}


def _get_nc(name):
    if name not in _NC_CACHE:
        _NC_CACHE[name] = {"corr": _build_corr_nc, "agg": _build_agg_nc}[name]()
    return _NC_CACHE[name]


_JIT_CACHE = {}


def _run_spmd(nc, in_maps):
    """run_bass_kernel_spmd's axon path with the jitted executable cached
    per-module, so repeat kernel() calls don't re-trace/re-compile."""
    import jax
    import numpy as _np
    from jax.experimental.shard_map import shard_map
    from jax.sharding import Mesh, PartitionSpec

    from concourse import bass2jax

    key = id(nc)
    if key not in _JIT_CACHE:
        bass2jax.install_neuronx_cc_hook()
        partition_name = (
            nc.partition_id_tensor.name if nc.partition_id_tensor else None
        )
        in_names, out_names, out_avals = [], [], []
        for alloc in nc.m.functions[0].allocations:
            if not isinstance(alloc, mybir.MemoryLocationSet):
                continue
            name = alloc.memorylocations[0].name
            if alloc.kind == "ExternalInput":
                if name != partition_name:
                    in_names.append(name)
            elif alloc.kind == "ExternalOutput":
                out_names.append(name)
                out_avals.append(
                    jax.core.ShapedArray(
                        tuple(alloc.tensor_shape), mybir.dt.np(alloc.dtype)
                    )
                )
        n_params = len(in_names)
        all_in_names = in_names + out_names
        if partition_name is not None:
            all_in_names = all_in_names + [partition_name]

        def _body(*args):
            operands = list(args)
            if partition_name is not None:
                operands.append(bass2jax.partition_id_tensor())
            outs = bass2jax._bass_exec_p.bind(
                *operands,
                out_avals=tuple(out_avals),
                in_names=tuple(all_in_names),
                out_names=tuple(out_names),
                lowering_input_output_aliases=(),
                sim_require_finite=True,
                sim_require_nnan=True,
                nc=nc,
            )
            return tuple(outs)

        devices = jax.devices()[:NCORES]
        mesh = Mesh(_np.asarray(devices), ("core",))
        n_outs = len(out_names)
        sharded = jax.jit(
            shard_map(
                _body,
                mesh=mesh,
                in_specs=(PartitionSpec("core"),) * (n_params + n_outs),
                out_specs=(PartitionSpec("core"),) * n_outs,
                check_rep=False,
            ),
            donate_argnums=tuple(range(n_params, n_params + n_outs)),
            keep_unused=True,
        )
        _JIT_CACHE[key] = (sharded, in_names, out_names, out_avals)

    sharded, in_names, out_names, out_avals = _JIT_CACHE[key]
    concat_in = [
        np.concatenate([np.asarray(m[name]) for m in in_maps], axis=0)
        for name in in_names
    ]
    concat_zeros = [
        np.zeros((NCORES * a.shape[0], *a.shape[1:]), a.dtype) for a in out_avals
    ]
    out_arrs = sharded(*concat_in, *concat_zeros)
    return [
        {
            name: np.asarray(out_arrs[i]).reshape(NCORES, *out_avals[i].shape)[c]
            for i, name in enumerate(out_names)
        }
        for c in range(NCORES)
    ]


def _run_spmd_safe(nc, in_maps):
    try:
        return _run_spmd(nc, in_maps)
    except Exception:
        from concourse.bass_utils import run_bass_kernel_spmd

        return run_bass_kernel_spmd(
            nc, in_maps, core_ids=list(range(NCORES))
        ).results


def kernel(queries, keys, values, attn_mask=None, **_unused):
    queries = np.asarray(queries)
    keys = np.asarray(keys)
    values = np.asarray(values)

    # ---- host prep: per-core sharded, time-last transposed q/k fp16 --------
    def _pack(x):
        # [B,N,L,H,E] -> [B,N,128,768] fp16, p-major (d = c*128+p, x = c*192+l)
        xt = x.transpose(0, 1, 3, 4, 2).reshape(B, N, 4, 128, L)
        return (
            np.ascontiguousarray(xt.transpose(0, 1, 3, 2, 4))
            .astype(np.float16)
            .reshape(B, N, 128, 768)
        )

    ktx = _pack(keys)
    qtx = _pack(queries)
    kqx = np.stack([ktx, qtx], axis=3)  # [B, N, 128, 2, 768]

    in_maps1 = []
    for i in range(NCORES):
        sl = slice(i * NLOC, (i + 1) * NLOC)
        in_maps1.append(
            {"kqx": np.ascontiguousarray(kqx[:, sl]).reshape(BN, 128, 2, 768)}
        )

    nc1 = _get_nc("corr")
    res1 = _run_spmd_safe(nc1, in_maps1)

    # ---- host: diag sums -> mean_value, top-k, softmax ---------------------
    # g5[core, quad, s, b4, u] fp16; c[tau] = sum_s G[s, (s+tau)%L]
    g_all = np.stack([r["g5"] for r in res1]).astype(np.float32)
    g2 = np.concatenate([g_all, g_all], axis=-1)  # [NC, 32, 192, 4, 384]
    st = g2.strides
    diag_view = np.lib.stride_tricks.as_strided(
        g2,
        shape=(NCORES, BN // 4, 4, L, L),  # [NC, quad, b4, s, tau]
        strides=(st[0], st[1], st[3], st[2] + st[4], st[4]),
    )
    c_all = diag_view.sum(axis=3, dtype=np.float64)  # [NC, quad, b4, tau]
    mean_value = (
        c_all.reshape(NCORES, B, NLOC, L).transpose(1, 0, 2, 3).reshape(B, N, L)
        / HE
    )
    z = mean_value.mean(axis=0)  # [N, L]
    # jax.lax.top_k semantics: descending, ties -> lowest index (stable)
    index = np.argsort(-z, axis=-1, kind="stable")[:, :TOPK]  # [N, K]
    w = np.take_along_axis(mean_value, index[None], axis=-1)  # [B, N, K]
    e = np.exp(w - w.max(axis=-1, keepdims=True))
    tmp_corr = e / e.sum(axis=-1, keepdims=True)  # [B, N, K]

    # ---- host: build A^T (shift matrices), shard v -------------------------
    # AT[b, n, t', tau] = w_j  where t' = (tau + d_j) % L
    pos = np.arange(L)
    rows = (pos[None, None, :] + index[:, :, None]) % L  # [N, K, L]
    AT = np.zeros((B, N, L, L), dtype=np.float32)
    bI = np.arange(B)[:, None, None, None]
    nI = np.arange(N)[None, :, None, None]
    AT[bI, nI, rows[None], pos[None, None, None, :]] = tmp_corr[:, :, :, None]
    AT = AT.astype(np.float16)

    v_flat = values.reshape(B, N, L, HD).astype(np.float16)

    in_maps2 = []
    for i in range(NCORES):
        sl = slice(i * NLOC, (i + 1) * NLOC)
        at_core = AT[:, sl].reshape(BN, L, L)
        # at6[quad, p, kc, b4, t]: t' = kc*96+p
        at_core = at_core.reshape(BN // 4, 4, 2, 96, L).transpose(0, 3, 2, 1, 4)
        v_core = v_flat[:, sl].reshape(BN // 4, 4, 2, 96, HD).transpose(
            0, 3, 2, 1, 4
        )
        in_maps2.append(
            {
                "at6": np.ascontiguousarray(at_core),
                "v6": np.ascontiguousarray(v_core),
            }
        )

    nc2 = _get_nc("agg")
    res2 = _run_spmd_safe(nc2, in_maps2)

    # o7[quad, p, half, cc, b4, l] fp16: out element
    # (4*quad+b4, hd=(half*2+cc)*128+p, tau=l)
    o_all = np.stack([r["o7"] for r in res2])  # [NC, BN/4, 128, 2, 2, 4, L]
    o_all = (
        o_all.astype(np.float32)
        .transpose(0, 1, 5, 3, 4, 2, 6)  # [NC, quad, b4, half, cc, p, l]
        .reshape(NCORES, BN, HD, L)
    )
    out = (
        o_all.transpose(0, 1, 3, 2)  # [NC, BN, L, HD]
        .reshape(NCORES, B, NLOC, L, H, D)
        .transpose(1, 0, 2, 3, 4, 5)
        .reshape(B, N, L, H, D)
    )
    return np.ascontiguousarray(out.astype(np.float32))
